# revision 1
# baseline (speedup 1.0000x reference)
"""Trainium2 Bass kernel for ExpanderLinearLayer (gather-mul-scatter_add).

Reformulation: out = input_ @ S + bias, where S[i, j] = sum of weight[k] over
all k with ind_in[k] == i and ind_out[k] == j.  S is built dense on the host
(52224 nnz into 1024x1024, ~0.5% of the device FLOPs) and the device runs a
dense fp32r matmul, data-parallel over the batch across 8 NeuronCores.

Per core (batch shard of 512 rows), the 1024-long contraction dim is split
into 8 chunks of 128.  Chunk k of the merged input tensor `xs` holds
[x_k | s_k] side by side so ONE DMA (one semaphore lane) delivers everything
the chunk-k matmuls need — engine instructions can carry only a single
sync-wait, so every instruction must depend on at most one semaphore.
Chunk 0 additionally carries the 8 per-m-tile bias columns.

  chunk k (k>0) at cols [8 + k*1536, 8 + (k+1)*1536):   [x_k | s_k]
  chunk 0 at cols [0, 8 + 1536):                        [bias | x_0 | s_0]
      x_k[p, n] = input_[c*512+n, k*128+p]   (n < 512)
      s_k[p, m] = S[k*128+p, m]              (m < 1024)
      bias[p, m] = bias[m*128+p]             (m < 8)
  o  [128, 8*512]:  o[p, m*512+n] = out[c*512+n, m*128+p]

Matmul (k outer, m inner): psum[m] += s_k[:, mblk].T @ x_k, fp32r (FP22
mantissa, full PE rate at N=512), accumulated over k in 8 PSUM banks, then
per-partition bias-add into one SBUF tile, one SWDGE DMA out.
"""

import os
import numpy as np

try:
    from concourse import bacc, bass, mybir
    from concourse.tile import TileContext
    from concourse.bass_utils import run_bass_kernel_spmd
except ImportError:  # fresh dir without PYTHONPATH
    import sys

    sys.path.insert(0, "/opt/trn_rl_repo")
    from concourse import bacc, bass, mybir
    from concourse.tile import TileContext
    from concourse.bass_utils import run_bass_kernel_spmd

P = 128
B = 4096
D = 1024
NCORES = 8
BS = B // NCORES      # 512 batch rows per core
KO = D // P           # 8 contraction chunks
MO = D // P           # 8 output tiles
CW = BS + D           # 1536 columns per merged chunk

F32 = mybir.dt.float32
F32R = mybir.dt.float32r

_NC_CACHE = {}
LAST_RESULTS = None


def _build_nc():
    # Bacc (not raw Bass): its compile() pass legalizes multi-wait
    # instructions (event semaphores, matmul waits moved to ldweights) —
    # TPB instructions encode only a single sync-wait.
    nc = bacc.Bacc("TRN2", target_bir_lowering=False)
    xs_d = nc.declare_dram_parameter("xs", [P, MO + KO * CW], F32R, isOutput=False)
    o_d = nc.declare_dram_parameter("o", [P, MO * BS], F32, isOutput=True)

    with TileContext(nc) as tc:
        with (
            tc.tile_pool(name="cs", bufs=1) as cpool,
            tc.tile_pool(name="bb", bufs=1) as bpool,
            tc.tile_pool(name="ob", bufs=1) as opool,
            tc.tile_pool(name="ps", bufs=1, space="PSUM") as pspool,
        ):
            chunks = []
            for k in range(KO):
                w = CW + MO if k == 0 else CW
                off = 0 if k == 0 else MO + k * CW
                ct = cpool.tile([P, w], F32R, tag=f"c{k}", name=f"c{k}")
                nc.sync.dma_start(ct, xs_d[:, off:off + w])
                chunks.append(ct)

            # bias columns live at the head of chunk 0
            bias_ap = chunks[0][:, :MO].bitcast(F32)

            def chunk_x(k):
                base = MO if k == 0 else 0
                return chunks[k][:, base:base + BS]

            def chunk_s(k, m):
                base = (MO if k == 0 else 0) + BS
                return chunks[k][:, base + m * P:base + (m + 1) * P]

            psums = [
                pspool.tile([P, BS], F32, tag=f"ps{m}", name=f"ps{m}")
                for m in range(MO)
            ]
            for k in range(KO):
                rhs = chunk_x(k)
                for m in range(MO):
                    nc.tensor.matmul(
                        psums[m],
                        lhsT=chunk_s(k, m),
                        rhs=rhs,
                        start=(k == 0),
                        stop=(k == KO - 1),
                    )

            out_sb = opool.tile([P, MO, BS], F32, tag="out")
            for m in range(MO):
                nc.vector.tensor_scalar_add(
                    out_sb[:, m], psums[m], bias_ap[:, m:m + 1]
                )
            # SWDGE: keeps the output DMA off the HWDGE semaphore lanes the
            # input chunks occupy (and off the tail drain's HW-lane budget).
            nc.gpsimd.dma_start(
                o_d[:, :].rearrange("p (m n) -> p m n", m=MO), out_sb[:]
            )

    nc.finalize()
    return nc


def _get_nc():
    if "nc" not in _NC_CACHE:
        _NC_CACHE["nc"] = _build_nc()
    return _NC_CACHE["nc"]


def kernel(input_, weight, bias, ind_in, ind_out):
    global LAST_RESULTS
    input_ = np.asarray(input_, dtype=np.float32)
    weight = np.asarray(weight, dtype=np.float32)
    bias = np.asarray(bias, dtype=np.float32)
    ind_in = np.asarray(ind_in, dtype=np.int64)
    ind_out = np.asarray(ind_out, dtype=np.int64)

    # Dense scatter matrix S.
    S = np.zeros((D, D), np.float32)
    np.add.at(S, (ind_in, ind_out), weight)
    b_l = np.ascontiguousarray(bias.reshape(MO, P).T)  # [128, 8]

    in_maps = []
    for c in range(NCORES):
        xT = input_[c * BS:(c + 1) * BS].T  # [1024, 512]
        xs_l = np.empty((P, MO + KO * CW), np.float32)
        xs_l[:, :MO] = b_l
        for k in range(KO):
            rows = slice(k * P, (k + 1) * P)
            off = MO + k * CW
            xs_l[:, off:off + BS] = xT[rows]
            xs_l[:, off + BS:off + CW] = S[rows]
        in_maps.append({"xs": xs_l})

    nc = _get_nc()
    res = run_bass_kernel_spmd(
        nc,
        in_maps,
        core_ids=list(range(NCORES)),
        trace=bool(int(os.environ.get("KERNEL_TRACE", "0"))),
    )
    LAST_RESULTS = res

    outs = []
    for c in range(NCORES):
        o = res.results[c]["o"]
        outT = o.reshape(P, MO, BS).transpose(1, 0, 2).reshape(D, BS)
        outs.append(outT.T)
    return np.ascontiguousarray(np.concatenate(outs, axis=0))



# revision 2
# speedup vs baseline: 1.3056x; 1.3056x over previous
"""Trainium2 Bass kernel for ExpanderLinearLayer (gather-mul-scatter_add).

Reformulation: out = input_ @ S + bias, where S[i, j] = sum of weight[k] over
all k with ind_in[k] == i and ind_out[k] == j.  S is built dense on the host
(52224 nnz into 1024x1024, ~0.5% of the device FLOPs) and the device runs a
dense bf16 matmul, data-parallel over the batch across 8 NeuronCores.

v2 (vs fp32r baseline at 44.1us):
  * all data bf16 (tolerance 2e-2 >> bf16's ~5e-3): halves DMA traffic
    (8.4MB -> 4.1MB/core) and runs the PE at full bf16 rate (~213ns/MM
    warm at N=512) instead of fp32 HIGH mode (~430ns/MM).
  * PE warm-up: ~12 tiny matmuls on a memset tile during the input-DMA
    head so the HAM clock-gate (1.2GHz cold -> 2.4GHz warm after ~3.4us
    of activity) warms before the real matmuls.
  * all 8 input chunks on the sync HWDGE ring (FIFO: chunk 0's data is
    not bandwidth-shared with later chunks, PE starts ~1.7us after T0).
  * tail: last two k-passes merged per-m ((6,m),(7,m) back to back) so
    psum[m] finalizes every ~0.43us; drains alternate vector/scalar
    (parallel PSUM banks) and 3 output DMAs ship finished thirds.

Per core (batch shard of 512 rows), the 1024-long contraction dim is split
into 8 chunks of 128.  Chunk k of the merged input tensor `xs` holds
[x_k | s_k] side by side so ONE DMA (one semaphore lane) delivers everything
the chunk-k matmuls need.  Chunk 0 additionally carries the 8 fp32 bias
columns (16 bf16 columns, bitcast on device).

  chunk k (k>0) at cols [16 + k*1536, 16 + (k+1)*1536):   [x_k | s_k]
  chunk 0 at cols [0, 16 + 1536):                         [bias | x_0 | s_0]
      x_k[p, n] = input_[c*512+n, k*128+p]   (n < 512)
      s_k[p, m] = S[k*128+p, m]              (m < 1024)
      bias[p, m] = bias[m*128+p]             (m < 8, fp32)
  o  [128, 8*512] bf16:  o[p, m*512+n] = out[c*512+n, m*128+p]
"""

import os
import numpy as np

try:
    from concourse import bacc, bass, mybir
    from concourse.tile import TileContext
    from concourse.bass_utils import run_bass_kernel_spmd
except ImportError:  # fresh dir without PYTHONPATH
    import sys

    sys.path.insert(0, "/opt/trn_rl_repo")
    from concourse import bacc, bass, mybir
    from concourse.tile import TileContext
    from concourse.bass_utils import run_bass_kernel_spmd

P = 128
B = 4096
D = 1024
NCORES = 8
BS = B // NCORES      # 512 batch rows per core
KO = D // P           # 8 contraction chunks
MO = D // P           # 8 output tiles
CW = BS + D           # 1536 columns per merged chunk
MOH = 2 * MO          # bf16 cols holding the fp32 bias at chunk-0 head
NWARM = 12            # PE warm-up matmuls during the DMA head

F32 = mybir.dt.float32
BF16 = mybir.dt.bfloat16
BF16_NP = mybir.dt.np(BF16)

_NC_CACHE = {}
LAST_RESULTS = None


def _build_nc():
    # Bacc (not raw Bass): its compile() pass legalizes multi-wait
    # instructions (event semaphores, matmul waits moved to ldweights) —
    # TPB instructions encode only a single sync-wait.
    nc = bacc.Bacc("TRN2", target_bir_lowering=False)
    xs_d = nc.declare_dram_parameter("xs", [P, MOH + KO * CW], BF16, isOutput=False)
    o_d = nc.declare_dram_parameter("o", [P, MO * BS], BF16, isOutput=True)

    with TileContext(nc) as tc:
        with (
            tc.tile_pool(name="cs", bufs=1) as cpool,
            tc.tile_pool(name="ob", bufs=1) as opool,
            tc.tile_pool(name="ps", bufs=1, space="PSUM") as pspool,
        ):
            # Single HWDGE ring (sync): FIFO ordering means chunk 0's
            # data drains at full HBM rate before later chunks start.
            chunks = []
            for k in range(KO):
                w = CW + MOH if k == 0 else CW
                off = 0 if k == 0 else MOH + k * CW
                ct = cpool.tile([P, w], BF16, tag=f"c{k}", name=f"c{k}")
                nc.sync.dma_start(ct, xs_d[:, off:off + w])
                chunks.append(ct)

            # fp32 bias columns live at the head of chunk 0
            bias_ap = chunks[0][:, :MOH].bitcast(F32)

            def chunk_x(k):
                base = MOH if k == 0 else 0
                return chunks[k][:, base:base + BS]

            def chunk_s(k, m):
                base = (MOH if k == 0 else 0) + BS
                return chunks[k][:, base + m * P:base + (m + 1) * P]

            psums = [
                pspool.tile([P, BS], F32, tag=f"ps{m}", name=f"ps{m}")
                for m in range(MO)
            ]
            out_sb = opool.tile([P, MO, BS], BF16, tag="out")

            # PE warm-up: HAM clock-gates a cold PE to 1.2GHz; ~3.4us of
            # activity unlocks 2.4GHz.  Start the activity window during
            # the chunk-0 DMA with throwaway matmuls on a memset tile.
            wu = cpool.tile([P, 64], BF16, tag="wu")
            nc.gpsimd.memset(wu[:, :], 0.0)
            for _ in range(NWARM):
                nc.tensor.matmul(
                    psums[0][:64, :64], lhsT=wu[:, :64], rhs=wu[:, :64],
                    start=True, stop=True,
                )

            # k-passes 0..5: psum[m] += s_k[m].T @ x_k
            for k in range(KO - 2):
                rhs = chunk_x(k)
                for m in range(MO):
                    nc.tensor.matmul(
                        psums[m],
                        lhsT=chunk_s(k, m),
                        rhs=rhs,
                        start=(k == 0),
                        stop=False,
                    )
            # merged tail passes 6+7: finalize psum[m] every ~0.43us and
            # drain it immediately; vector/scalar alternate so the two
            # PSUM readers run in parallel on different banks.
            for m in range(MO):
                nc.tensor.matmul(
                    psums[m], lhsT=chunk_s(KO - 2, m), rhs=chunk_x(KO - 2),
                    start=False, stop=False,
                )
                nc.tensor.matmul(
                    psums[m], lhsT=chunk_s(KO - 1, m), rhs=chunk_x(KO - 1),
                    start=False, stop=True,
                )
                if m % 2 == 0:
                    nc.vector.tensor_scalar_add(
                        out_sb[:, m], psums[m], bias_ap[:, m:m + 1]
                    )
                else:
                    nc.scalar.add(out_sb[:, m], psums[m], bias_ap[:, m:m + 1])

            # ship finished thirds; sync ring is idle after the input
            # doorbells, and the last DMA (2 blocks) keeps the tail short.
            out_r = o_d[:, :].rearrange("p (m n) -> p m n", m=MO)
            nc.sync.dma_start(out_r[:, 0:3], out_sb[:, 0:3])
            nc.sync.dma_start(out_r[:, 3:6], out_sb[:, 3:6])
            nc.sync.dma_start(out_r[:, 6:8], out_sb[:, 6:8])

    nc.finalize()
    return nc


def _get_nc():
    if "nc" not in _NC_CACHE:
        _NC_CACHE["nc"] = _build_nc()
    return _NC_CACHE["nc"]


def kernel(input_, weight, bias, ind_in, ind_out):
    global LAST_RESULTS
    input_ = np.asarray(input_, dtype=np.float32)
    weight = np.asarray(weight, dtype=np.float32)
    bias = np.asarray(bias, dtype=np.float32)
    ind_in = np.asarray(ind_in, dtype=np.int64)
    ind_out = np.asarray(ind_out, dtype=np.int64)

    # Dense scatter matrix S.
    S = np.zeros((D, D), np.float32)
    np.add.at(S, (ind_in, ind_out), weight)
    S16 = S.astype(BF16_NP)
    # fp32 bias [128, 8] viewed as bf16 [128, 16] for the merged DMA
    b_l = np.ascontiguousarray(bias.reshape(MO, P).T).view(BF16_NP)

    in_maps = []
    for c in range(NCORES):
        xT = np.ascontiguousarray(
            input_[c * BS:(c + 1) * BS].T
        ).astype(BF16_NP)  # [1024, 512]
        xs_l = np.empty((P, MOH + KO * CW), BF16_NP)
        xs_l[:, :MOH] = b_l
        for k in range(KO):
            rows = slice(k * P, (k + 1) * P)
            off = MOH + k * CW
            xs_l[:, off:off + BS] = xT[rows]
            xs_l[:, off + BS:off + CW] = S16[rows]
        in_maps.append({"xs": xs_l})

    nc = _get_nc()
    res = run_bass_kernel_spmd(
        nc,
        in_maps,
        core_ids=list(range(NCORES)),
        trace=bool(int(os.environ.get("KERNEL_TRACE", "0"))),
    )
    LAST_RESULTS = res

    outs = []
    for c in range(NCORES):
        o = np.asarray(res.results[c]["o"], dtype=np.float32)
        outT = o.reshape(P, MO, BS).transpose(1, 0, 2).reshape(D, BS)
        outs.append(outT.T)
    return np.ascontiguousarray(np.concatenate(outs, axis=0))
